# revision 13
# baseline (speedup 1.0000x reference)
"""Trainium2 Bass kernel for CrossAttention1D.

Strategy: data-parallel over batch B=8 (one batch per NeuronCore).
Per core, everything is fused in one program:
  - transpose xq/xk/xv via PE (fp32 has no DMA-transpose path)
  - Q^T/K^T (dim-major) and V (token-major) projections in fp32r
  - attn_avg = (Q K^T) * SCALE / H as a single full-dim matmul
    (sum of per-head dot products == full 1024-dim dot product)
  - per head: scores computed TRANSPOSED [k_tok, q_tok] so softmax'd
    probabilities feed PV directly; exp on ScalarE from PSUM; softmax
    denominator folded into PV as a concurrent col-tiled ones-matmul;
    normalization applied to the tiny PV output instead of the scores
  - final projection from the accumulated out^T (already in lhsT layout)
SCALE is folded into Wq/bq on the host.
"""

import sys

if "/opt/trn_rl_repo" not in sys.path:
    sys.path.insert(0, "/opt/trn_rl_repo")

import numpy as np

import concourse.bacc as bacc
import concourse.mybir as mybir
from concourse.bass_utils import run_bass_kernel_spmd
from concourse.masks import make_identity
from concourse.tile import TileContext

F32 = mybir.dt.float32
F32R = mybir.dt.float32r
F16 = mybir.dt.float16
AF = mybir.ActivationFunctionType

N = 1024   # tokens
C = 1024   # model dim
H = 16     # heads
D = 64     # head dim
P = 128    # partitions
NT = N // P   # 8 token tiles
CT = C // P   # 8 contraction tiles
NCH = 2       # 512-wide chunks per 1024
SCALE = D ** -0.5
B = 8


def _r(ap):
    return ap.bitcast(F32R)


def _emit(nc, reps=1):
    xq = nc.dram_tensor("xq", [N, C], F32, kind="ExternalInput")
    xk = nc.dram_tensor("xk", [N, C], F32, kind="ExternalInput")
    xv = nc.dram_tensor("xv", [N, C], F32, kind="ExternalInput")
    Wq = nc.dram_tensor("Wq", [C, C], F32, kind="ExternalInput")
    Wk = nc.dram_tensor("Wk", [C, C], F32, kind="ExternalInput")
    Wv = nc.dram_tensor("Wv", [C, C], F32, kind="ExternalInput")
    Wp = nc.dram_tensor("Wp", [C, C], F32, kind="ExternalInput")
    bq = nc.dram_tensor("bq", [C], F32, kind="ExternalInput")
    bk = nc.dram_tensor("bk", [C], F32, kind="ExternalInput")
    bv = nc.dram_tensor("bv", [C], F32, kind="ExternalInput")
    bp = nc.dram_tensor("bp", [C], F32, kind="ExternalInput")
    out = nc.dram_tensor("out", [N, C], F32, kind="ExternalOutput")
    attn_avg = nc.dram_tensor("attn_avg", [N, N], F32, kind="ExternalOutput")
    OT_d = nc.dram_tensor("OT_scratch", [C, N], F32R)  # out^T bounce

    with TileContext(nc) as tc:
      for _rep in range(reps):
        with (
            tc.tile_pool(name="persist", bufs=1) as persist,
            tc.tile_pool(name="ps", bufs=3, space="PSUM") as ps,
            tc.tile_pool(name="po", bufs=1, space="PSUM") as po,
        ):
            ident = persist.tile([P, P], F32, tag="ident")
            make_identity(nc, ident[:])
            ones_f = persist.tile([P, 1], F32, tag="ones_f")
            nc.vector.memset(ones_f[:], 1.0)
            ones16 = persist.tile([P, 1], F16, tag="ones16")
            nc.vector.tensor_copy(ones16[:], ones_f[:])
            identr = persist.tile([P, P], F32R, tag="identr")
            nc.vector.tensor_copy(identr[:], ident[:])

            # biases: bq/bk as [P, CT] (per-partition for dim-major adds)
            bq_t = persist.tile([P, CT], F32, tag="bq")
            bk_t = persist.tile([P, CT], F32, tag="bk")
            for mt in range(CT):
                nc.sync.dma_start(out=bq_t[:, mt : mt + 1], in_=bq[mt * P : (mt + 1) * P, None])
                nc.sync.dma_start(out=bk_t[:, mt : mt + 1], in_=bk[mt * P : (mt + 1) * P, None])
            # bv/bp broadcast to all partitions (free-dim adds)
            bv_bc = persist.tile([P, C], F32, tag="bv")
            bp_bc = persist.tile([P, C], F32, tag="bp")
            nc.sync.dma_start(out=bv_bc[0:1, :], in_=bv[None, :])
            nc.sync.dma_start(out=bp_bc[0:1, :], in_=bp[None, :])
            nc.gpsimd.partition_broadcast(bv_bc[:], bv_bc[0:1, :], channels=P)
            nc.gpsimd.partition_broadcast(bp_bc[:], bp_bc[0:1, :], channels=P)

            # persistent activations
            QT = persist.tile([P, CT, N], F32R, tag="QT")    # Q^T: [dim, tok]
            KT = persist.tile([P, CT, N], F32R, tag="KT")    # K^T: [dim, tok]
            V = persist.tile([P, NT, C], F16, tag="V")      # V:   [tok, dim]

            # ================= phase 1: transposes + projections =============
            with (
                tc.tile_pool(name="xT", bufs=1) as xTp,
                tc.tile_pool(name="stg", bufs=2) as stg,
                tc.tile_pool(name="wmt", bufs=3) as wmtp,
                tc.tile_pool(name="wv", bufs=1) as wvp,
            ):
                def transpose_in(x_dram):
                    xT = xTp.tile([P, CT, N], F32R, tag="xT")
                    for tt in range(NT):
                        slab = stg.tile([P, C], F32R, tag="stg")
                        nc.sync.dma_start(out=slab[:], in_=x_dram[tt * P : (tt + 1) * P, :].bitcast(F32R))
                        pst = ps.tile([P, N], F32R, tag="ps")
                        for ci in range(CT):
                            nc.tensor.transpose(
                                pst[:, ci * P : (ci + 1) * P],
                                slab[:, ci * P : (ci + 1) * P],
                                identr[:],
                            )
                        src = pst[:].rearrange("p (ci q) -> p ci q", ci=CT)
                        if tt % 2 == 0:
                            nc.vector.tensor_copy(xT[:, :, tt * P : (tt + 1) * P], src)
                        else:
                            nc.scalar.activation(xT[:, :, tt * P : (tt + 1) * P], src, AF.Copy)
                    return xT

                def proj_dim_major(xT, W_dram, bias_t, dst):
                    # dst[:, mt, :] = (W^T x^T)[dim-tile mt, tok] + bias
                    for mt in range(CT):
                        wmt = wmtp.tile([P, CT, P], F32R, tag="wmt")
                        nc.sync.dma_start(
                            out=wmt[:],
                            in_=W_dram[:, mt * P : (mt + 1) * P].bitcast(F32R).rearrange(
                                "(ct p) m -> p ct m", p=P
                            ),
                        )
                        pp = ps.tile([P, N], F32, tag="ps")
                        for nch in range(NCH):
                            for ct in range(CT):
                                nc.tensor.matmul(
                                    pp[:, nch * 512 : (nch + 1) * 512],
                                    _r(wmt[:, ct, :]),
                                    _r(xT[:, ct, nch * 512 : (nch + 1) * 512]),
                                    start=(ct == 0),
                                    stop=(ct == CT - 1),
                                )
                        nc.scalar.activation(
                            dst[:, mt, :], pp[:], AF.Identity, bias=bias_t[:, mt : mt + 1]
                        )

                xqT = transpose_in(xq)
                proj_dim_major(xqT, Wq, bq_t, QT)
                xkT = transpose_in(xk)
                proj_dim_major(xkT, Wk, bk_t, KT)
                xvT = transpose_in(xv)
                # V token-major: V[tt, :] = x_v @ Wv + bv
                for half in range(2):
                    wvh = wvp.tile([P, CT, 512], F32R, tag="wv")
                    nc.sync.dma_start(
                        out=wvh[:],
                        in_=Wv[:, half * 512 : (half + 1) * 512].bitcast(F32R).rearrange(
                            "(ct p) m -> p ct m", p=P
                        ),
                    )
                    for tt in range(NT):
                        pv = ps.tile([P, 512], F32, tag="ps")
                        for ct in range(CT):
                            nc.tensor.matmul(
                                pv[:],
                                _r(xvT[:, ct, tt * P : (tt + 1) * P]),
                                _r(wvh[:, ct, :]),
                                start=(ct == 0),
                                stop=(ct == CT - 1),
                            )
                        nc.vector.tensor_add(
                            V[:, tt, half * 512 : (half + 1) * 512],
                            pv[:],
                            bv_bc[:, half * 512 : (half + 1) * 512],
                        )

            # ============ phase 2: attn_avg + attention + final ==============
            with (
                tc.tile_pool(name="pT", bufs=4) as pTp,
                tc.tile_pool(name="misc", bufs=2) as misc,
                tc.tile_pool(name="outsb", bufs=2) as outsb,
                tc.tile_pool(name="wp", bufs=1) as wpp,
                tc.tile_pool(name="otmt", bufs=3) as otmtp,
            ):
                # attn_avg: full-dim Q K^T, scaled by 1/H (SCALE folded in Wq)
                for qt in range(NT):
                    pa = ps.tile([P, N], F32, tag="ps")
                    for nch in range(NCH):
                        for ct in range(CT):
                            nc.tensor.matmul(
                                pa[:, nch * 512 : (nch + 1) * 512],
                                _r(QT[:, ct, qt * P : (qt + 1) * P]),
                                _r(KT[:, ct, nch * 512 : (nch + 1) * 512]),
                                start=(ct == 0),
                                stop=(ct == CT - 1),
                            )
                    av = outsb.tile([P, N], F32, tag="o")
                    nc.vector.tensor_scalar_mul(av[:], pa[:], 1.0 / H)
                    nc.sync.dma_start(out=attn_avg[qt * P : (qt + 1) * P, :], in_=av[:])

                # per-head attention
                for h in range(H):
                    ct_h = h // 2
                    po_h = (h % 2) * D
                    psum_o = po.tile([D + 1, N], F32, tag="po")
                    for kt in range(NT):
                        psum_s = ps.tile([P, N], F32, tag="ps")
                        for nch in range(NCH):
                            nc.tensor.matmul(
                                psum_s[:, nch * 512 : (nch + 1) * 512],
                                _r(KT[po_h : po_h + D, ct_h, kt * P : (kt + 1) * P]),
                                _r(QT[po_h : po_h + D, ct_h, nch * 512 : (nch + 1) * 512]),
                                start=True,
                                stop=True,
                            )
                        pt = pTp.tile([P, N], F16, tag="pT")
                        nc.scalar.activation(pt[:], psum_s[:], AF.Exp)
                        for nch in range(NCH):
                            nc.tensor.matmul(
                                psum_o[0:D, nch * 512 : (nch + 1) * 512],
                                V[:, kt, h * D : (h + 1) * D],
                                pt[:, nch * 512 : (nch + 1) * 512],
                                start=(kt == 0),
                                stop=(kt == NT - 1),
                                tile_position=(0, 0),
                                skip_group_check=True,
                            )
                            nc.tensor.matmul(
                                psum_o[D : D + 1, nch * 512 : (nch + 1) * 512],
                                ones16[:],
                                pt[:, nch * 512 : (nch + 1) * 512],
                                start=(kt == 0),
                                stop=(kt == NT - 1),
                                tile_position=(0, 64),
                                skip_group_check=True,
                            )
                    rc = misc.tile([1, N], F32, tag="recip")
                    nc.vector.reciprocal(rc[0:1, :], psum_o[D : D + 1, :])
                    bc = misc.tile([D, N], F32, tag="bcast")
                    nc.gpsimd.partition_broadcast(bc[:], rc[0:1, :], channels=D)
                    otb = misc.tile([D, N], F32R, tag="otb")
                    nc.vector.tensor_mul(otb[:], psum_o[0:D, :], bc[:])
                    nc.sync.dma_start(out=OT_d[h * D : (h + 1) * D, :], in_=otb[:])

                # final projection: out = OT^T @ Wp + bp
                wp_t = wpp.tile([P, CT, C], F32R, tag="wp")
                nc.sync.dma_start(
                    out=wp_t[:], in_=Wp[:, :].bitcast(F32R).rearrange("(ct p) m -> p ct m", p=P)
                )
                for qt in range(NT):
                    ot_mt = otmtp.tile([P, CT, P], F32R, tag="otmt")
                    nc.sync.dma_start(
                        out=ot_mt[:],
                        in_=OT_d[:, qt * P : (qt + 1) * P].rearrange("(ct p) m -> p ct m", p=P),
                    )
                    pf = ps.tile([P, N], F32, tag="ps")
                    for nch in range(NCH):
                        for ct in range(CT):
                            nc.tensor.matmul(
                                pf[:, nch * 512 : (nch + 1) * 512],
                                _r(ot_mt[:, ct, :]),
                                _r(wp_t[:, ct, nch * 512 : (nch + 1) * 512]),
                                start=(ct == 0),
                                stop=(ct == CT - 1),
                            )
                    ot = outsb.tile([P, C], F32, tag="o")
                    nc.vector.tensor_add(ot[:], pf[:], bp_bc[:])
                    nc.sync.dma_start(out=out[qt * P : (qt + 1) * P, :], in_=ot[:])

    return nc


LAST_RESULT = None
_NC_CACHE = {}


def _get_nc(reps=1):
    if reps not in _NC_CACHE:
        nc = bacc.Bacc("TRN2", target_bir_lowering=False, debug=False)
        _emit(nc, reps)
        nc.compile()
        _NC_CACHE[reps] = nc
    return _NC_CACHE[reps]


def kernel(xq, xk, xv, Wq, bq, Wk, bk, Wv, bv, Wp, bp, **_ignored):
    nc = _get_nc()
    Wq_s = np.asarray(Wq, np.float32) * np.float32(SCALE)
    bq_s = np.asarray(bq, np.float32) * np.float32(SCALE)
    common = {
        "Wq": np.ascontiguousarray(Wq_s),
        "bq": np.ascontiguousarray(bq_s),
        "Wk": np.ascontiguousarray(np.asarray(Wk, np.float32)),
        "bk": np.ascontiguousarray(np.asarray(bk, np.float32)),
        "Wv": np.ascontiguousarray(np.asarray(Wv, np.float32)),
        "bv": np.ascontiguousarray(np.asarray(bv, np.float32)),
        "Wp": np.ascontiguousarray(np.asarray(Wp, np.float32)),
        "bp": np.ascontiguousarray(np.asarray(bp, np.float32)),
    }
    in_maps = []
    for b in range(B):
        m = dict(common)
        m["xq"] = np.ascontiguousarray(np.asarray(xq[b], np.float32))
        m["xk"] = np.ascontiguousarray(np.asarray(xk[b], np.float32))
        m["xv"] = np.ascontiguousarray(np.asarray(xv[b], np.float32))
        in_maps.append(m)
    res = run_bass_kernel_spmd(nc, in_maps, list(range(B)))
    global LAST_RESULT
    LAST_RESULT = res
    out = np.stack([res.results[b]["out"] for b in range(B)])
    attn_avg = np.stack([res.results[b]["attn_avg"] for b in range(B)])
    return out, attn_avg


# revision 14
# speedup vs baseline: 1.0576x; 1.0576x over previous
"""Trainium2 Bass kernel for CrossAttention1D.

Strategy: data-parallel over batch B=8 (one batch per NeuronCore).
Per core, everything is fused in one program:
  - transpose xq/xk/xv via PE (fp32 has no DMA-transpose path)
  - Q^T/K^T (dim-major) and V (token-major) projections in fp32r
  - attn_avg = (Q K^T) * SCALE / H as a single full-dim matmul
    (sum of per-head dot products == full 1024-dim dot product)
  - per head: scores computed TRANSPOSED [k_tok, q_tok] so softmax'd
    probabilities feed PV directly; exp on ScalarE from PSUM; softmax
    denominator folded into PV as a concurrent col-tiled ones-matmul;
    normalization applied to the tiny PV output instead of the scores
  - final projection from the accumulated out^T (already in lhsT layout)
SCALE is folded into Wq/bq on the host.
"""

import sys

if "/opt/trn_rl_repo" not in sys.path:
    sys.path.insert(0, "/opt/trn_rl_repo")

import numpy as np

import concourse.bacc as bacc
import concourse.mybir as mybir
from concourse.bass_utils import run_bass_kernel_spmd
from concourse.masks import make_identity
from concourse.tile import TileContext

F32 = mybir.dt.float32
F32R = mybir.dt.float32r
F16 = mybir.dt.float16
AF = mybir.ActivationFunctionType

N = 1024   # tokens
C = 1024   # model dim
H = 16     # heads
D = 64     # head dim
P = 128    # partitions
NT = N // P   # 8 token tiles
CT = C // P   # 8 contraction tiles
NCH = 2       # 512-wide chunks per 1024
SCALE = D ** -0.5
B = 8


def _r(ap):
    return ap.bitcast(F32R)


def _emit(nc, reps=1):
    xq = nc.dram_tensor("xq", [N, C], F32, kind="ExternalInput")
    xk = nc.dram_tensor("xk", [N, C], F32, kind="ExternalInput")
    xv = nc.dram_tensor("xv", [N, C], F32, kind="ExternalInput")
    Wq = nc.dram_tensor("Wq", [C, C], F32, kind="ExternalInput")
    Wk = nc.dram_tensor("Wk", [C, C], F32, kind="ExternalInput")
    Wv = nc.dram_tensor("Wv", [C, C], F32, kind="ExternalInput")
    Wp = nc.dram_tensor("Wp", [C, C], F32, kind="ExternalInput")
    bq = nc.dram_tensor("bq", [C], F32, kind="ExternalInput")
    bk = nc.dram_tensor("bk", [C], F32, kind="ExternalInput")
    bv = nc.dram_tensor("bv", [C], F32, kind="ExternalInput")
    bp = nc.dram_tensor("bp", [C], F32, kind="ExternalInput")
    out = nc.dram_tensor("out", [N, C], F32, kind="ExternalOutput")
    attn_avg = nc.dram_tensor("attn_avg", [N, N], F32, kind="ExternalOutput")

    with TileContext(nc) as tc:
      for _rep in range(reps):
        with (
            tc.tile_pool(name="persist", bufs=1) as persist,
            tc.tile_pool(name="ps", bufs=3, space="PSUM") as ps,
            tc.tile_pool(name="po", bufs=1, space="PSUM") as po,
        ):
            ident = persist.tile([P, P], F32, tag="ident")
            make_identity(nc, ident[:])
            ones_f = persist.tile([P, 1], F32, tag="ones_f")
            nc.vector.memset(ones_f[:], 1.0)
            ones16 = persist.tile([P, 1], F16, tag="ones16")
            nc.vector.tensor_copy(ones16[:], ones_f[:])
            identr = persist.tile([P, P], F32R, tag="identr")
            nc.vector.tensor_copy(identr[:], ident[:])

            # biases: bq/bk as [P, CT] (per-partition for dim-major adds)
            bq_t = persist.tile([P, CT], F32, tag="bq")
            bk_t = persist.tile([P, CT], F32, tag="bk")
            for mt in range(CT):
                nc.sync.dma_start(out=bq_t[:, mt : mt + 1], in_=bq[mt * P : (mt + 1) * P, None])
                nc.sync.dma_start(out=bk_t[:, mt : mt + 1], in_=bk[mt * P : (mt + 1) * P, None])
            # bv/bp broadcast to all partitions (free-dim adds)
            bv_bc = persist.tile([P, C], F32, tag="bv")
            bp_bc = persist.tile([P, C], F32, tag="bp")
            nc.sync.dma_start(out=bv_bc[0:1, :], in_=bv[None, :])
            nc.sync.dma_start(out=bp_bc[0:1, :], in_=bp[None, :])
            nc.gpsimd.partition_broadcast(bv_bc[:], bv_bc[0:1, :], channels=P)
            nc.gpsimd.partition_broadcast(bp_bc[:], bp_bc[0:1, :], channels=P)

            # persistent activations
            QT = persist.tile([P, CT, N], F32R, tag="QT")    # Q^T: [dim, tok]
            KT = persist.tile([P, CT, N], F32R, tag="KT")    # K^T: [dim, tok]
            V = persist.tile([P, NT, C], F16, tag="V")      # V:   [tok, dim]
            OT = persist.tile([P, CT, N], F32R, tag="OT")   # out^T: [dim, tok]

            # ================= phase 1: transposes + projections =============
            with (
                tc.tile_pool(name="xT", bufs=1) as xTp,
                tc.tile_pool(name="stg", bufs=2) as stg,
                tc.tile_pool(name="wmt", bufs=3) as wmtp,
                tc.tile_pool(name="wv", bufs=1) as wvp,
            ):
                def transpose_in(x_dram):
                    xT = xTp.tile([P, CT, N], F32R, tag="xT")
                    for tt in range(NT):
                        slab = stg.tile([P, C], F32R, tag="stg")
                        nc.sync.dma_start(out=slab[:], in_=x_dram[tt * P : (tt + 1) * P, :].bitcast(F32R))
                        pst = ps.tile([P, N], F32R, tag="ps")
                        for ci in range(CT):
                            nc.tensor.transpose(
                                pst[:, ci * P : (ci + 1) * P],
                                slab[:, ci * P : (ci + 1) * P],
                                identr[:],
                            )
                        src = pst[:].rearrange("p (ci q) -> p ci q", ci=CT)
                        if tt % 2 == 0:
                            nc.vector.tensor_copy(xT[:, :, tt * P : (tt + 1) * P], src)
                        else:
                            nc.scalar.activation(xT[:, :, tt * P : (tt + 1) * P], src, AF.Copy)
                    return xT

                def proj_dim_major(xT, W_dram, bias_t, dst):
                    # dst[:, mt, :] = (W^T x^T)[dim-tile mt, tok] + bias
                    for mt in range(CT):
                        wmt = wmtp.tile([P, CT, P], F32R, tag="wmt")
                        nc.sync.dma_start(
                            out=wmt[:],
                            in_=W_dram[:, mt * P : (mt + 1) * P].bitcast(F32R).rearrange(
                                "(ct p) m -> p ct m", p=P
                            ),
                        )
                        pp = ps.tile([P, N], F32, tag="ps")
                        for nch in range(NCH):
                            for ct in range(CT):
                                nc.tensor.matmul(
                                    pp[:, nch * 512 : (nch + 1) * 512],
                                    _r(wmt[:, ct, :]),
                                    _r(xT[:, ct, nch * 512 : (nch + 1) * 512]),
                                    start=(ct == 0),
                                    stop=(ct == CT - 1),
                                )
                        nc.scalar.activation(
                            dst[:, mt, :], pp[:], AF.Identity, bias=bias_t[:, mt : mt + 1]
                        )

                xqT = transpose_in(xq)
                proj_dim_major(xqT, Wq, bq_t, QT)
                xkT = transpose_in(xk)
                proj_dim_major(xkT, Wk, bk_t, KT)
                xvT = transpose_in(xv)
                # V token-major: V[tt, :] = x_v @ Wv + bv
                for half in range(2):
                    wvh = wvp.tile([P, CT, 512], F32R, tag="wv")
                    nc.sync.dma_start(
                        out=wvh[:],
                        in_=Wv[:, half * 512 : (half + 1) * 512].bitcast(F32R).rearrange(
                            "(ct p) m -> p ct m", p=P
                        ),
                    )
                    for tt in range(NT):
                        pv = ps.tile([P, 512], F32, tag="ps")
                        for ct in range(CT):
                            nc.tensor.matmul(
                                pv[:],
                                _r(xvT[:, ct, tt * P : (tt + 1) * P]),
                                _r(wvh[:, ct, :]),
                                start=(ct == 0),
                                stop=(ct == CT - 1),
                            )
                        nc.vector.tensor_add(
                            V[:, tt, half * 512 : (half + 1) * 512],
                            pv[:],
                            bv_bc[:, half * 512 : (half + 1) * 512],
                        )

            # ============ phase 2: attn_avg + attention + final ==============
            with (
                tc.tile_pool(name="pT", bufs=4) as pTp,
                tc.tile_pool(name="misc", bufs=2) as misc,
                tc.tile_pool(name="outsb", bufs=2) as outsb,
                tc.tile_pool(name="wp", bufs=1) as wpp,
            ):
                # attn_avg: full-dim Q K^T, scaled by 1/H (SCALE folded in Wq)
                for qt in range(NT):
                    pa = ps.tile([P, N], F32, tag="ps")
                    for nch in range(NCH):
                        for ct in range(CT):
                            nc.tensor.matmul(
                                pa[:, nch * 512 : (nch + 1) * 512],
                                _r(QT[:, ct, qt * P : (qt + 1) * P]),
                                _r(KT[:, ct, nch * 512 : (nch + 1) * 512]),
                                start=(ct == 0),
                                stop=(ct == CT - 1),
                            )
                    av = outsb.tile([P, N], F32, tag="o")
                    nc.vector.tensor_scalar_mul(av[:], pa[:], 1.0 / H)
                    nc.sync.dma_start(out=attn_avg[qt * P : (qt + 1) * P, :], in_=av[:])

                # per-head attention
                for h in range(H):
                    ct_h = h // 2
                    po_h = (h % 2) * D
                    psum_o = po.tile([D + 1, N], F32, tag="po")
                    for kt in range(NT):
                        psum_s = ps.tile([P, N], F32, tag="ps")
                        for nch in range(NCH):
                            nc.tensor.matmul(
                                psum_s[:, nch * 512 : (nch + 1) * 512],
                                _r(KT[po_h : po_h + D, ct_h, kt * P : (kt + 1) * P]),
                                _r(QT[po_h : po_h + D, ct_h, nch * 512 : (nch + 1) * 512]),
                                start=True,
                                stop=True,
                            )
                        pt = pTp.tile([P, N], F16, tag="pT")
                        nc.scalar.activation(pt[:], psum_s[:], AF.Exp)
                        for nch in range(NCH):
                            nc.tensor.matmul(
                                psum_o[0:D, nch * 512 : (nch + 1) * 512],
                                V[:, kt, h * D : (h + 1) * D],
                                pt[:, nch * 512 : (nch + 1) * 512],
                                start=(kt == 0),
                                stop=(kt == NT - 1),
                                tile_position=(0, 0),
                                skip_group_check=True,
                            )
                            nc.tensor.matmul(
                                psum_o[D : D + 1, nch * 512 : (nch + 1) * 512],
                                ones16[:],
                                pt[:, nch * 512 : (nch + 1) * 512],
                                start=(kt == 0),
                                stop=(kt == NT - 1),
                                tile_position=(0, 64),
                                skip_group_check=True,
                            )
                    rc = misc.tile([1, N], F32, tag="recip")
                    nc.vector.reciprocal(rc[0:1, :], psum_o[D : D + 1, :])
                    bc = misc.tile([D, N], F32, tag="bcast")
                    nc.gpsimd.partition_broadcast(bc[:], rc[0:1, :], channels=D)
                    nc.vector.tensor_mul(OT[po_h : po_h + D, ct_h, :], psum_o[0:D, :], bc[:])

                # final projection: out = OT^T @ Wp + bp
                wp_t = wpp.tile([P, CT, C], F32R, tag="wp")
                nc.sync.dma_start(
                    out=wp_t[:], in_=Wp[:, :].bitcast(F32R).rearrange("(ct p) m -> p ct m", p=P)
                )
                for qt in range(NT):
                    pf = ps.tile([P, N], F32, tag="ps")
                    for nch in range(NCH):
                        for ct in range(CT):
                            nc.tensor.matmul(
                                pf[:, nch * 512 : (nch + 1) * 512],
                                _r(OT[:, ct, qt * P : (qt + 1) * P]),
                                _r(wp_t[:, ct, nch * 512 : (nch + 1) * 512]),
                                start=(ct == 0),
                                stop=(ct == CT - 1),
                            )
                    ot = outsb.tile([P, C], F32, tag="o")
                    nc.vector.tensor_add(ot[:], pf[:], bp_bc[:])
                    nc.sync.dma_start(out=out[qt * P : (qt + 1) * P, :], in_=ot[:])

    return nc


LAST_RESULT = None
_NC_CACHE = {}


def _get_nc(reps=1):
    if reps not in _NC_CACHE:
        nc = bacc.Bacc("TRN2", target_bir_lowering=False, debug=False)
        _emit(nc, reps)
        nc.compile()
        _NC_CACHE[reps] = nc
    return _NC_CACHE[reps]


def kernel(xq, xk, xv, Wq, bq, Wk, bk, Wv, bv, Wp, bp, **_ignored):
    nc = _get_nc()
    Wq_s = np.asarray(Wq, np.float32) * np.float32(SCALE)
    bq_s = np.asarray(bq, np.float32) * np.float32(SCALE)
    common = {
        "Wq": np.ascontiguousarray(Wq_s),
        "bq": np.ascontiguousarray(bq_s),
        "Wk": np.ascontiguousarray(np.asarray(Wk, np.float32)),
        "bk": np.ascontiguousarray(np.asarray(bk, np.float32)),
        "Wv": np.ascontiguousarray(np.asarray(Wv, np.float32)),
        "bv": np.ascontiguousarray(np.asarray(bv, np.float32)),
        "Wp": np.ascontiguousarray(np.asarray(Wp, np.float32)),
        "bp": np.ascontiguousarray(np.asarray(bp, np.float32)),
    }
    in_maps = []
    for b in range(B):
        m = dict(common)
        m["xq"] = np.ascontiguousarray(np.asarray(xq[b], np.float32))
        m["xk"] = np.ascontiguousarray(np.asarray(xk[b], np.float32))
        m["xv"] = np.ascontiguousarray(np.asarray(xv[b], np.float32))
        in_maps.append(m)
    res = run_bass_kernel_spmd(nc, in_maps, list(range(B)))
    global LAST_RESULT
    LAST_RESULT = res
    out = np.stack([res.results[b]["out"] for b in range(B)])
    attn_avg = np.stack([res.results[b]["attn_avg"] for b in range(B)])
    return out, attn_avg
